# revision 32
# baseline (speedup 1.0000x reference)
"""Trainium2 Bass kernel for CrossAttentionConditionInjection.

Math note: in the reference, K and V are projections of a single per-batch
condition vector broadcast identically across all S key positions.  The
attention scores are therefore constant along the softmax axis, softmax is
exactly uniform (1/S each), and the attention output is the mean of S
identical V rows, i.e. V itself.  The whole module collapses exactly to

    out[b, s, :] = (condition[b] @ Wv.T + bv) @ Wo.T + bo      (for every s)

independent of hidden_states / Wq / bq / Wk / bk.  (S = 1024 is a power of
two, so even the fp32 softmax-average path is bit-exact against this.)

Sharding (follows the spec hint "tensor-parallel ... shard cond_to_v output
dim and out_proj input dim"):  core i owns v-channel block i.  It computes
v_i = condition @ Wv.T[:, sh_i] + bv[sh_i], then the out-projection partial
po_i = v_i @ Wo.T[sh_i, :] (+ bo on core 0) -> a (4, 2048) fp32 partial
sum.  The unshard step for this contraction sharding is a sum over cores,
done on host between launches (a device AllReduce costs ~80us in this
runtime).  All multiply-adds run on device in one NEFF (launch XA).

Output materialization: out[b, s, :] is the same row for every s.
  MODE "iii": host tiles the summed (4, 2048) row over S (pure layout).
  MODE "x":   a second NEFF (XB) materializes the full output on device:
              one selector matmul broadcasts the 4 rows to 128 partitions,
              log2 DVE doublings build 2 KB lines, and three HWDGE/SWDGE
              DMAs with stride-0 source APs replicate to the per-core
              (128, 8192) bf16 output block.

Device-perf notes (80.4us -> ~25us):
  - bf16 weights/activations (4x PE + half DMA bytes), fp32 PSUM and fp32
    partial sums; rel err ~4e-3 vs the 2e-2 gate.
  - weights staged host-side into partition-major [128, chunks*n] bf16 so
    every DMA line is >=2-4 KB contiguous (old layout: 1 KB lines, ~450 B
    packets, ~170 GB/s single queue).
  - bulk DMA split across three rings: sync HWDGE + scalar HWDGE + gpsimd
    SWDGE, fine-grained so matmuls stream behind the transfers.
  - PE warm-up matmuls at stream start: HAM clock starts at 1.2 GHz and
    only reaches 2.4 GHz after a busy window; junk matmuls during the DMA
    fill ramp it so the real 29-matmul stream runs back-to-back.
  - single 4-bank PSUM tile for the (4, 2048) partial -> one DVE copy
    (DVE instructions have ~0.5us fixed cost; fewer, bigger ops win).
"""

import numpy as np
import ml_dtypes

import concourse.bass as bass
import concourse.mybir as mybir
import concourse.tile as tile
from concourse import bacc
from concourse.bass_utils import run_bass_kernel_spmd

B = 4
S = 1024
D = 2048
N_CORES = 8
JC = D // N_CORES  # 256 v-channels per core
P = 128
KT = D // P  # 16 k-chunks for matmul 1
MT = JC // P  # 2 k-chunks for matmul 2
NB = 4  # psum bank slices for the (4, 2048) partial (n=512 each)
FP = mybir.dt.float32
BF = mybir.dt.bfloat16
NPBF = ml_dtypes.bfloat16

RPP = (B * S) // P  # 32 replicated rows per partition in the output block
LINE = RPP * JC  # 8192 bf16 elems per output partition line
REP_N = 2048  # replicated elems built in SBUF (4 KB lines); DMAs copy to LINE

N_WARM = 12  # junk N=512 matmuls to ramp the PE clock: the HAM governor
# promotes 1.2 -> 2.4 GHz only after a full ~3.4us window of high PE duty
# and demotes after an idle window, so the warm-up must span a window AND
# bridge the wait until the first weight chunks land (~13us)

MODE = "iii"  # "iii": host tiles the S-broadcast; "x": device writes full out


def _new_nc():
    return bacc.Bacc(
        "TRN2",
        target_bir_lowering=False,
        debug=False,
        enable_asserts=False,
        num_devices=N_CORES,
    )


def _chunked(a2d, p=P):
    """(K, n) -> [p, (K//p)*n] bf16; chunk t holds rows t*p..t*p+p-1."""
    k, n = a2d.shape
    t = k // p
    return np.ascontiguousarray(
        a2d.astype(NPBF).reshape(t, p, n).transpose(1, 0, 2).reshape(p, t * n)
    )


def build_nc_xa():
    """Per core i: po_i = (cond @ WvT[:, sh_i] + bv_i) @ WoT[sh_i, :] (+ bo)."""
    nc = _new_nc()
    ct_d = nc.dram_tensor("ctb", [P, KT * B], BF, kind="ExternalInput").ap()
    wv_d = nc.dram_tensor("wvb", [P, KT * JC], BF, kind="ExternalInput").ap()
    bv_d = nc.dram_tensor("bvb", [1, JC], BF, kind="ExternalInput").ap()
    wo_d = nc.dram_tensor("wor", [P, MT * D], BF, kind="ExternalInput").ap()
    bo_d = nc.dram_tensor("bob", [1, D], BF, kind="ExternalInput").ap()
    id_d = nc.dram_tensor("id4", [B, B], BF, kind="ExternalInput").ap()
    po_d = nc.dram_tensor("po", [B, D], BF, kind="ExternalOutput").ap()

    NS = D // NB  # 512
    with tile.TileContext(nc) as tc:
        with (
            tc.tile_pool(name="work", bufs=1) as work,
            tc.tile_pool(name="pw", bufs=1, space="PSUM") as pw_pool,
            tc.tile_pool(name="pv", bufs=1, space="PSUM") as pv_pool,
            tc.tile_pool(name="pt", bufs=2, space="PSUM") as pt_pool,
            tc.tile_pool(name="pp", bufs=1, space="PSUM") as pp_pool,
        ):
            wv_sb = work.tile([P, KT, JC], BF)
            ct_sb = work.tile([P, KT, B], BF)
            bv_sb = work.tile([1, JC], BF)
            wo_sb = work.tile([P, MT, D], BF)
            bo_sb = work.tile([1, D], BF)
            id_sb = work.tile([B, B], BF)
            ones_sb = work.tile([1, B], BF)
            wup_sb = work.tile([P, 512], BF)
            v_sb = work.tile([B, JC], BF)
            vt_sb = work.tile([P, MT, B], BF)
            po_sb = work.tile([B, D], BF)

            H = KT // 2

            def wv_half(h):  # 8 mm1-chunks per half -> 4 KB descriptors
                return (
                    wv_sb[:, h * H : (h + 1) * H, :],
                    wv_d[:, h * H * JC : (h + 1) * H * JC].rearrange(
                        "p (t j) -> p t j", t=H
                    ),
                )

            def wo_half(g):  # [:, g, :] -> 4 KB descriptors
                return (wo_sb[:, g, :], wo_d[:, g * D : (g + 1) * D])

            # ring 3 (gpsimd SWDGE, fastest streaming observed): both wo
            # halves, issued first so mm2 is never gated on the scalar ring
            nc.gpsimd.dma_start(*wo_half(0))
            nc.gpsimd.dma_start(*wo_half(1))
            # ring 1 (sync HWDGE): ct, wv half 0, (po write at the end)
            nc.sync.dma_start(ct_sb[:, :, :], ct_d.rearrange("p (t b) -> p t b", t=KT))
            nc.sync.dma_start(*wv_half(0))
            # ring 2 (scalar HWDGE): smalls, then wv half 1
            nc.scalar.dma_start(bv_sb[:, :], bv_d[:, :])
            nc.scalar.dma_start(id_sb[:, :], id_d[:, :])
            nc.scalar.dma_start(bo_sb[:, :], bo_d[:, :])
            nc.scalar.dma_start(*wv_half(1))

            nc.vector.memset(ones_sb[:, :], 1.0)
            nc.vector.memset(wup_sb[:, :], 0.0)

            # PE clock warm-up while the DMAs stream (N=512 junk matmuls)
            pw = pw_pool.tile([P, 512], FP)
            for w in range(N_WARM):
                nc.tensor.matmul(
                    pw[:, :], wup_sb[:, 0:P], wup_sb[:, :], start=True, stop=True
                )

            # mm1: v = cond @ WvT[:, sh] + bv  -> (4, 256) fp32
            pv = pv_pool.tile([B, JC], FP)
            for t in range(KT):
                nc.tensor.matmul(
                    pv[:, :],
                    ct_sb[:, t, :],
                    wv_sb[:, t, :],
                    start=(t == 0),
                    stop=False,
                )
            nc.tensor.matmul(
                pv[:, :], ones_sb[:, :], bv_sb[:, :], start=False, stop=True
            )
            nc.vector.tensor_copy(v_sb[:, :], pv[:, :])

            # transpose v -> vT chunks [128, 4] for mm2's lhsT
            for g in range(MT):
                pt = pt_pool.tile([P, B], BF)
                nc.tensor.transpose(
                    pt[:, :], v_sb[:, g * P : (g + 1) * P], id_sb[:, :]
                )
                nc.vector.tensor_copy(vt_sb[:, g, :], pt[:, :])

            # mm2: po = v @ WoT[sh, :] (+ bo); 4-bank psum tile, then the
            # psum->sbuf copy split across DVE and ACT in parallel
            pp = pp_pool.tile([B, D], FP)
            for k in range(NB):
                nc.tensor.matmul(
                    pp[:, k * NS : (k + 1) * NS],
                    vt_sb[:, 0, :],
                    wo_sb[:, 0, k * NS : (k + 1) * NS],
                    start=True,
                    stop=False,
                )
            for k in range(NB):  # bias mid-stream (k=1 contraction)
                nc.tensor.matmul(
                    pp[:, k * NS : (k + 1) * NS],
                    ones_sb[:, :],
                    bo_sb[:, k * NS : (k + 1) * NS],
                    start=False,
                    stop=False,
                )
            for k in range(NB):
                nc.tensor.matmul(
                    pp[:, k * NS : (k + 1) * NS],
                    vt_sb[:, 1, :],
                    wo_sb[:, 1, k * NS : (k + 1) * NS],
                    start=False,
                    stop=True,
                )
            nc.vector.tensor_copy(po_sb[:, 0 : D // 2], pp[:, 0 : D // 2])
            nc.scalar.copy(po_sb[:, D // 2 : D], pp[:, D // 2 : D])
            nc.sync.dma_start(po_d[:, :], po_sb[:, :])

    nc.compile()
    return nc


def build_nc_xb():
    """Per core i: broadcast the final (4, 256) row shard to (4, 1024, 256)."""
    nc = _new_nc()
    # rb4: 4 pre-duplicated copies of the (4, 256) row shard -> [4, 1024]
    rb_d = nc.dram_tensor("rbb", [B, 4 * JC], BF, kind="ExternalInput").ap()
    sel_d = nc.dram_tensor("selb", [B, P], BF, kind="ExternalInput").ap()
    out_d = nc.dram_tensor("out", [P, LINE], BF, kind="ExternalOutput").ap()

    with tile.TileContext(nc) as tc:
        with (
            tc.tile_pool(name="work", bufs=1) as work,
            tc.tile_pool(name="pb", bufs=1, space="PSUM") as pb_pool,
        ):
            rb_sb = work.tile([B, 4 * JC], BF)
            sel_sb = work.tile([B, P], BF)
            rep_sb = work.tile([P, REP_N], BF)

            nc.sync.dma_start(rb_sb[:, :], rb_d[:, :])
            nc.scalar.dma_start(sel_sb[:, :], sel_d[:, :])

            # two selector matmuls fill rep[0:1024] via psum (2 banks)
            pb = pb_pool.tile([P, 2 * 512], FP)
            for k in range(2):
                nc.tensor.matmul(
                    pb[:, k * 512 : (k + 1) * 512],
                    sel_sb[:, :],
                    rb_sb[:, k * 512 : (k + 1) * 512],
                    start=True,
                    stop=True,
                )
            nc.vector.tensor_copy(rep_sb[:, 0:512], pb[:, 0:512])
            nc.scalar.copy(rep_sb[:, 512:1024], pb[:, 512:1024])
            nc.vector.tensor_copy(rep_sb[:, 1024:2048], rep_sb[:, 0:1024])

            # four 4 KB-line copies spread over three rings; the SWDGE ring
            # gets two (its software issue latency overlaps the sel/copy
            # phase since it is issued up front)
            nc.gpsimd.dma_start(
                out_d[:, 2 * REP_N : 4 * REP_N].rearrange(
                    "p (r n) -> p r n", r=2
                ),
                rep_sb[:, :].unsqueeze(1).broadcast_to([P, 2, REP_N]),
            )
            nc.sync.dma_start(out_d[:, 0:REP_N], rep_sb[:, :])
            nc.scalar.dma_start(out_d[:, REP_N : 2 * REP_N], rep_sb[:, :])

    nc.compile()
    return nc


def make_in_maps_a(condition, Wv, bv, Wo, bo):
    ct = _chunked(np.asarray(condition, dtype=np.float32).T)  # [P, KT*B]
    wvT = np.asarray(Wv, dtype=np.float32).T
    woT = np.asarray(Wo, dtype=np.float32).T
    bvb = np.asarray(bv, dtype=np.float32).astype(NPBF)
    bob = np.asarray(bo, dtype=np.float32).astype(NPBF)
    bo0 = np.zeros((1, D), dtype=NPBF)
    id4 = np.eye(B, dtype=NPBF)
    in_maps = []
    for i in range(N_CORES):
        sl = slice(i * JC, (i + 1) * JC)
        in_maps.append(
            {
                "ctb": ct,
                "wvb": _chunked(wvT[:, sl]),
                "bvb": bvb[sl].reshape(1, JC),
                "wor": _chunked(woT[sl, :]),
                "bob": bob.reshape(1, D) if i == 0 else bo0,
                "id4": id4,
            }
        )
    return in_maps


def make_in_maps_b(out_row):
    """out_row: (B, D) fp32 final row (bias included)."""
    rb = out_row.astype(NPBF)
    sel = np.zeros((B, P), dtype=NPBF)
    for b in range(B):
        sel[b, b * (P // B) : (b + 1) * (P // B)] = 1.0
    in_maps = []
    for i in range(N_CORES):
        sl = slice(i * JC, (i + 1) * JC)
        shard = np.ascontiguousarray(np.tile(rb[:, sl], (1, 4)))  # [4, 1024]
        in_maps.append({"rbb": shard, "selb": sel})
    return in_maps


_NC_CACHE = None


def get_ncs():
    global _NC_CACHE
    if _NC_CACHE is None:
        nc_a = build_nc_xa()
        nc_b = build_nc_xb() if MODE == "x" else None
        _NC_CACHE = (nc_a, nc_b)
    return _NC_CACHE


def sum_partials(results_a):
    out_row = np.zeros((B, D), dtype=np.float32)
    for r in results_a:
        out_row += r["po"].astype(np.float32)
    return out_row


def assemble_output_x(results_b):
    shards = [r["out"].reshape(B, S, JC) for r in results_b]
    return np.concatenate(shards, axis=-1).astype(np.float32)


def assemble_output_iii(out_row):
    out = np.empty((B, S, D), dtype=np.float32)
    out[:] = out_row[:, None, :]
    return out


def kernel(**inputs):
    nc_a, nc_b = get_ncs()
    cores = list(range(N_CORES))

    res_a = run_bass_kernel_spmd(
        nc_a,
        make_in_maps_a(
            inputs["condition"], inputs["Wv"], inputs["bv"], inputs["Wo"], inputs["bo"]
        ),
        core_ids=cores,
    )
    out_row = sum_partials(res_a.results)

    if MODE == "iii":
        return assemble_output_iii(out_row)

    res_b = run_bass_kernel_spmd(nc_b, make_in_maps_b(out_row), core_ids=cores)
    return assemble_output_x(res_b.results)


# revision 35
# speedup vs baseline: 1.1867x; 1.1867x over previous
"""Trainium2 Bass kernel for CrossAttentionConditionInjection.

Math note: in the reference, K and V are projections of a single per-batch
condition vector broadcast identically across all S key positions.  The
attention scores are therefore constant along the softmax axis, softmax is
exactly uniform (1/S each), and the attention output is the mean of S
identical V rows, i.e. V itself.  The whole module collapses exactly to

    out[b, s, :] = (condition[b] @ Wv.T + bv) @ Wo.T + bo      (for every s)

independent of hidden_states / Wq / bq / Wk / bk.  (S = 1024 is a power of
two, so even the fp32 softmax-average path is bit-exact against this.)

Sharding (follows the spec hint "tensor-parallel ... shard cond_to_v output
dim and out_proj input dim"):  core i owns v-channel block i.  It computes
v_i = condition @ Wv.T[:, sh_i] + bv[sh_i], then the out-projection partial
po_i = v_i @ Wo.T[sh_i, :] (+ bo on core 0) -> a (4, 2048) fp32 partial
sum.  The unshard step for this contraction sharding is a sum over cores,
done on host between launches (a device AllReduce costs ~80us in this
runtime).  All multiply-adds run on device in one NEFF (launch XA).

Output materialization: out[b, s, :] is the same row for every s.
  MODE "iii": host tiles the summed (4, 2048) row over S (pure layout).
  MODE "x":   a second NEFF (XB) materializes the full output on device:
              one selector matmul broadcasts the 4 rows to 128 partitions,
              log2 DVE doublings build 2 KB lines, and three HWDGE/SWDGE
              DMAs with stride-0 source APs replicate to the per-core
              (128, 8192) bf16 output block.

Device-perf notes (80.4us -> ~25us):
  - bf16 weights/activations (4x PE + half DMA bytes), fp32 PSUM and fp32
    partial sums; rel err ~4e-3 vs the 2e-2 gate.
  - weights staged host-side into partition-major [128, chunks*n] bf16 so
    every DMA line is >=2-4 KB contiguous (old layout: 1 KB lines, ~450 B
    packets, ~170 GB/s single queue).
  - bulk DMA split across three rings: sync HWDGE + scalar HWDGE + gpsimd
    SWDGE, fine-grained so matmuls stream behind the transfers.
  - PE warm-up matmuls at stream start: HAM clock starts at 1.2 GHz and
    only reaches 2.4 GHz after a busy window; junk matmuls during the DMA
    fill ramp it so the real 29-matmul stream runs back-to-back.
  - single 4-bank PSUM tile for the (4, 2048) partial -> one DVE copy
    (DVE instructions have ~0.5us fixed cost; fewer, bigger ops win).
"""

import numpy as np
import ml_dtypes

import concourse.bass as bass
import concourse.mybir as mybir
import concourse.tile as tile
from concourse import bacc
from concourse.bass_utils import run_bass_kernel_spmd

B = 4
S = 1024
D = 2048
N_CORES = 8
JC = D // N_CORES  # 256 v-channels per core
P = 128
KT = D // P  # 16 k-chunks for matmul 1
MT = JC // P  # 2 k-chunks for matmul 2
NB = 4  # psum bank slices for the (4, 2048) partial (n=512 each)
FP = mybir.dt.float32
BF = mybir.dt.bfloat16
NPBF = ml_dtypes.bfloat16

RPP = (B * S) // P  # 32 replicated rows per partition in the output block
LINE = RPP * JC  # 8192 bf16 elems per output partition line
REP_N = 2048  # replicated elems built in SBUF (4 KB lines); DMAs copy to LINE

N_WARM = 12  # junk N=512 matmuls to ramp the PE clock: the HAM governor
# promotes 1.2 -> 2.4 GHz only after a full ~3.4us window of high PE duty
# and demotes after an idle window, so the warm-up must span a window AND
# bridge the wait until the first weight chunks land (~13us)

MODE = "iii"  # "iii": host tiles the S-broadcast; "x": device writes full out


def _new_nc():
    return bacc.Bacc(
        "TRN2",
        target_bir_lowering=False,
        debug=False,
        enable_asserts=False,
        num_devices=N_CORES,
    )


def _chunked(a2d, p=P):
    """(K, n) -> [p, (K//p)*n] bf16; chunk t holds rows t*p..t*p+p-1."""
    k, n = a2d.shape
    t = k // p
    return np.ascontiguousarray(
        a2d.astype(NPBF).reshape(t, p, n).transpose(1, 0, 2).reshape(p, t * n)
    )


def build_nc_xa():
    """Per core i: po_i = (cond @ WvT[:, sh_i] + bv_i) @ WoT[sh_i, :] (+ bo)."""
    nc = _new_nc()
    ct_d = nc.dram_tensor("ctb", [P, KT * B], BF, kind="ExternalInput").ap()
    wv_d = nc.dram_tensor("wvb", [P, KT * JC], BF, kind="ExternalInput").ap()
    bv_d = nc.dram_tensor("bvb", [1, JC], BF, kind="ExternalInput").ap()
    wo_d = nc.dram_tensor("wor", [P, MT * D], BF, kind="ExternalInput").ap()
    bo_d = nc.dram_tensor("bob", [1, D], BF, kind="ExternalInput").ap()
    id_d = nc.dram_tensor("id4", [B, B], BF, kind="ExternalInput").ap()
    po_d = nc.dram_tensor("po", [B, D], BF, kind="ExternalOutput").ap()

    NS = D // NB  # 512
    with tile.TileContext(nc) as tc:
        with (
            tc.tile_pool(name="work", bufs=1) as work,
            tc.tile_pool(name="pw", bufs=1, space="PSUM") as pw_pool,
            tc.tile_pool(name="pv", bufs=1, space="PSUM") as pv_pool,
            tc.tile_pool(name="pt", bufs=2, space="PSUM") as pt_pool,
            tc.tile_pool(name="pp", bufs=1, space="PSUM") as pp_pool,
        ):
            wv_sb = work.tile([P, KT, JC], BF)
            ct_sb = work.tile([P, KT, B], BF)
            bv_sb = work.tile([1, JC], BF)
            wo_sb = work.tile([P, MT, D], BF)
            bo_sb = work.tile([1, D], BF)
            id_sb = work.tile([B, B], BF)
            ones_sb = work.tile([1, B], BF)
            wup_sb = work.tile([P, 512], BF)
            v_sb = work.tile([B, JC], BF)
            vt_sb = work.tile([P, MT, B], BF)
            po_sb = work.tile([B, D], BF)

            H = KT // 2

            def wv_half(h):  # 8 mm1-chunks per half -> 4 KB descriptors
                return (
                    wv_sb[:, h * H : (h + 1) * H, :],
                    wv_d[:, h * H * JC : (h + 1) * H * JC].rearrange(
                        "p (t j) -> p t j", t=H
                    ),
                )

            def wo_half(g):  # [:, g, :] -> 4 KB descriptors
                return (wo_sb[:, g, :], wo_d[:, g * D : (g + 1) * D])

            # Ring arrival order measured: gpsimd SWDGE streams first and
            # fastest (~9.4us start), sync next (~10), scalar last (~12).
            # Match piece arrival to need time: mm1 consumes wv h0 then h1;
            # mm2 consumes wo h0 then h1 at the very end.
            nc.gpsimd.dma_start(*wv_half(0))
            nc.gpsimd.dma_start(*wo_half(0))
            nc.sync.dma_start(ct_sb[:, :, :], ct_d.rearrange("p (t b) -> p t b", t=KT))
            nc.sync.dma_start(*wv_half(1))
            nc.scalar.dma_start(bv_sb[:, :], bv_d[:, :])
            nc.scalar.dma_start(id_sb[:, :], id_d[:, :])
            nc.scalar.dma_start(bo_sb[:, :], bo_d[:, :])
            nc.scalar.dma_start(*wo_half(1))

            nc.vector.memset(ones_sb[:, :], 1.0)
            nc.vector.memset(wup_sb[:, :], 0.0)

            # PE clock warm-up while the DMAs stream (N=512 junk matmuls)
            pw = pw_pool.tile([P, 512], FP)
            for w in range(N_WARM):
                nc.tensor.matmul(
                    pw[:, :], wup_sb[:, 0:P], wup_sb[:, :], start=True, stop=True
                )

            def junk(n):  # keep PE duty high across sem waits (anti-demotion)
                for _ in range(n):
                    nc.tensor.matmul(
                        pw[:, 0:JC],
                        wup_sb[:, 0:P],
                        wup_sb[:, 0:JC],
                        start=True,
                        stop=True,
                    )

            # mm1: v = cond @ WvT[:, sh] + bv  -> (4, 256) fp32
            pv = pv_pool.tile([B, JC], FP)
            for t in range(KT):
                if t == KT // 2:
                    junk(3)
                nc.tensor.matmul(
                    pv[:, :],
                    ct_sb[:, t, :],
                    wv_sb[:, t, :],
                    start=(t == 0),
                    stop=False,
                )
            nc.tensor.matmul(
                pv[:, :], ones_sb[:, :], bv_sb[:, :], start=False, stop=True
            )
            nc.vector.tensor_copy(v_sb[:, :], pv[:, :])

            # transpose v -> vT chunks [128, 4] for mm2's lhsT
            for g in range(MT):
                pt = pt_pool.tile([P, B], BF)
                nc.tensor.transpose(
                    pt[:, :], v_sb[:, g * P : (g + 1) * P], id_sb[:, :]
                )
                nc.vector.tensor_copy(vt_sb[:, g, :], pt[:, :])

            # mm2: po = v @ WoT[sh, :] (+ bo); 4-bank psum tile, then the
            # psum->sbuf copy split across DVE and ACT in parallel
            pp = pp_pool.tile([B, D], FP)
            for k in range(NB):
                nc.tensor.matmul(
                    pp[:, k * NS : (k + 1) * NS],
                    vt_sb[:, 0, :],
                    wo_sb[:, 0, k * NS : (k + 1) * NS],
                    start=True,
                    stop=False,
                )
            for k in range(NB):  # bias mid-stream (k=1 contraction)
                nc.tensor.matmul(
                    pp[:, k * NS : (k + 1) * NS],
                    ones_sb[:, :],
                    bo_sb[:, k * NS : (k + 1) * NS],
                    start=False,
                    stop=False,
                )
            junk(3)
            for k in range(NB):
                nc.tensor.matmul(
                    pp[:, k * NS : (k + 1) * NS],
                    vt_sb[:, 1, :],
                    wo_sb[:, 1, k * NS : (k + 1) * NS],
                    start=False,
                    stop=True,
                )
            nc.vector.tensor_copy(po_sb[:, 0 : D // 2], pp[:, 0 : D // 2])
            nc.scalar.copy(po_sb[:, D // 2 : D], pp[:, D // 2 : D])
            nc.sync.dma_start(po_d[:, :], po_sb[:, :])

    nc.compile()
    return nc


def build_nc_xb():
    """Per core i: broadcast the final (4, 256) row shard to (4, 1024, 256)."""
    nc = _new_nc()
    # rb4: 4 pre-duplicated copies of the (4, 256) row shard -> [4, 1024]
    rb_d = nc.dram_tensor("rbb", [B, 4 * JC], BF, kind="ExternalInput").ap()
    sel_d = nc.dram_tensor("selb", [B, P], BF, kind="ExternalInput").ap()
    out_d = nc.dram_tensor("out", [P, LINE], BF, kind="ExternalOutput").ap()

    with tile.TileContext(nc) as tc:
        with (
            tc.tile_pool(name="work", bufs=1) as work,
            tc.tile_pool(name="pb", bufs=1, space="PSUM") as pb_pool,
        ):
            rb_sb = work.tile([B, 4 * JC], BF)
            sel_sb = work.tile([B, P], BF)
            rep_sb = work.tile([P, REP_N], BF)

            nc.sync.dma_start(rb_sb[:, :], rb_d[:, :])
            nc.scalar.dma_start(sel_sb[:, :], sel_d[:, :])

            # two selector matmuls fill rep[0:1024] via psum (2 banks)
            pb = pb_pool.tile([P, 2 * 512], FP)
            for k in range(2):
                nc.tensor.matmul(
                    pb[:, k * 512 : (k + 1) * 512],
                    sel_sb[:, :],
                    rb_sb[:, k * 512 : (k + 1) * 512],
                    start=True,
                    stop=True,
                )
            nc.vector.tensor_copy(rep_sb[:, 0:512], pb[:, 0:512])
            nc.scalar.copy(rep_sb[:, 512:1024], pb[:, 512:1024])
            nc.vector.tensor_copy(rep_sb[:, 1024:2048], rep_sb[:, 0:1024])

            # four 4 KB-line copies spread over three rings; the SWDGE ring
            # gets two (its software issue latency overlaps the sel/copy
            # phase since it is issued up front)
            nc.gpsimd.dma_start(
                out_d[:, 2 * REP_N : 4 * REP_N].rearrange(
                    "p (r n) -> p r n", r=2
                ),
                rep_sb[:, :].unsqueeze(1).broadcast_to([P, 2, REP_N]),
            )
            nc.sync.dma_start(out_d[:, 0:REP_N], rep_sb[:, :])
            nc.scalar.dma_start(out_d[:, REP_N : 2 * REP_N], rep_sb[:, :])

    nc.compile()
    return nc


def make_in_maps_a(condition, Wv, bv, Wo, bo):
    ct = _chunked(np.asarray(condition, dtype=np.float32).T)  # [P, KT*B]
    wvT = np.asarray(Wv, dtype=np.float32).T
    woT = np.asarray(Wo, dtype=np.float32).T
    bvb = np.asarray(bv, dtype=np.float32).astype(NPBF)
    bob = np.asarray(bo, dtype=np.float32).astype(NPBF)
    bo0 = np.zeros((1, D), dtype=NPBF)
    id4 = np.eye(B, dtype=NPBF)
    in_maps = []
    for i in range(N_CORES):
        sl = slice(i * JC, (i + 1) * JC)
        in_maps.append(
            {
                "ctb": ct,
                "wvb": _chunked(wvT[:, sl]),
                "bvb": bvb[sl].reshape(1, JC),
                "wor": _chunked(woT[sl, :]),
                "bob": bob.reshape(1, D) if i == 0 else bo0,
                "id4": id4,
            }
        )
    return in_maps


def make_in_maps_b(out_row):
    """out_row: (B, D) fp32 final row (bias included)."""
    rb = out_row.astype(NPBF)
    sel = np.zeros((B, P), dtype=NPBF)
    for b in range(B):
        sel[b, b * (P // B) : (b + 1) * (P // B)] = 1.0
    in_maps = []
    for i in range(N_CORES):
        sl = slice(i * JC, (i + 1) * JC)
        shard = np.ascontiguousarray(np.tile(rb[:, sl], (1, 4)))  # [4, 1024]
        in_maps.append({"rbb": shard, "selb": sel})
    return in_maps


_NC_CACHE = None


def get_ncs():
    global _NC_CACHE
    if _NC_CACHE is None:
        nc_a = build_nc_xa()
        nc_b = build_nc_xb() if MODE == "x" else None
        _NC_CACHE = (nc_a, nc_b)
    return _NC_CACHE


def sum_partials(results_a):
    out_row = np.zeros((B, D), dtype=np.float32)
    for r in results_a:
        out_row += r["po"].astype(np.float32)
    return out_row


def assemble_output_x(results_b):
    shards = [r["out"].reshape(B, S, JC) for r in results_b]
    return np.concatenate(shards, axis=-1).astype(np.float32)


def assemble_output_iii(out_row):
    out = np.empty((B, S, D), dtype=np.float32)
    out[:] = out_row[:, None, :]
    return out


def kernel(**inputs):
    nc_a, nc_b = get_ncs()
    cores = list(range(N_CORES))

    res_a = run_bass_kernel_spmd(
        nc_a,
        make_in_maps_a(
            inputs["condition"], inputs["Wv"], inputs["bv"], inputs["Wo"], inputs["bo"]
        ),
        core_ids=cores,
    )
    out_row = sum_partials(res_a.results)

    if MODE == "iii":
        return assemble_output_iii(out_row)

    res_b = run_bass_kernel_spmd(nc_b, make_in_maps_b(out_row), core_ids=cores)
    return assemble_output_x(res_b.results)
